# revision 1
# baseline (speedup 1.0000x reference)
"""ChannelSELayer (global-avg-pool -> MLP -> sigmoid -> top-2 channel gather).

Full-input contract: kernel(**inputs) takes the complete tensors and returns
the complete output. Internally shards across 8 NeuronCores:

  core i -> (batch b = i//2, spatial half h = i%2, output rank r = i%2)

Per core:
  1. reduce its spatial half of x[b] (all 64 channels)       [32 MiB read]
  2. pair AllReduce of the [64] channel sums                 [256 B]
  3. replicated tiny MLP + top-2 (PE matmuls, DVE max8)      [~us]
  4. gather channel row idx[r] of x[b] via dynamic-offset DMA [1 MiB r + 1 MiB w]

Odd cores receive x[b] spatially rolled by S/2 so the SPMD program can always
reduce columns [0, S/2); the host rolls their gathered output row back.

Written in raw Bass (explicit blocks + semaphores): the toolchain's codegen
rejects instructions with more than one attached sync wait, so all waits are
standalone wait_ge instructions ahead of the op. Cross-partition data
movement for the tiny [64] vectors goes through PE matmuls against a
selection constant so every DRAM touch is a single contiguous line (a
[128,1] partition-scattered DMA costs ~26 us in 4-byte descriptors).
"""

import numpy as np
from contextlib import ExitStack

import concourse.bass as bass
import concourse.mybir as mybir
from concourse.bass_utils import run_bass_kernel_spmd

F32 = mybir.dt.float32
U32 = mybir.dt.uint32

B = 4
C = 64
S = 64 * 64 * 64  # flattened spatial (D*H*W)
R = 2
N_CORES = 8
LEAKY_SLOPE = 0.01


def build_bass(
    c=C,
    s=S,
    tile_f=1024,
    n_cores=N_CORES,
    use_collective=True,
    nslots=12,
    gather_chunks=8,  # per ring
    static_gather_row=None,  # debug: int -> bypass dynamic indexing
    # "err"-mode OOB notifications wedge the exec unit at this tensor size
    # (NRT_EXEC_UNIT_UNRECOVERABLE); the index is structurally in [0, c),
    # so skip mode never actually triggers.
    gather_bounds="skip_entire_dma",
    gather_engine="rings",  # "rings" (SP+ACT HWDGE) or "gpsimd" (SWDGE)
):
    """Build the SPMD Bass module (identical program on every core).

    Inputs (per core): x [c, s], w1t [c, c] (= W1.T / s), b1 [c, 1],
    w2ta [c+1, c] (= [W2.T; b2]), rsel [1, 8] one-hot uint32 rank select,
    sel [128, c] (= tile(eye(c), (q, 1))) channel-sum selection matrix.
    Output: out [s] = x[top_r_channel, :].
    """
    q = 128 // c  # channel sub-chunks so the reduction uses 128 partitions
    assert c * q == 128
    sh = s // 2 if use_collective else s  # spatial slice reduced on this core
    pp = sh // q  # elements per partition
    ntiles = max(1, pp // tile_f)
    tile_f = pp // ntiles
    assert tile_f * ntiles == pp

    nc = bass.Bass(num_devices=n_cores)
    x = nc.declare_dram_parameter("x", [c, s], F32, isOutput=False)
    w1t = nc.declare_dram_parameter("w1t", [c, c], F32, isOutput=False)
    b1 = nc.declare_dram_parameter("b1", [c, 1], F32, isOutput=False)
    w2ta = nc.declare_dram_parameter("w2ta", [c + 1, c], F32, isOutput=False)
    rsel = nc.declare_dram_parameter("rsel", [1, 8], U32, isOutput=False)
    sel = nc.declare_dram_parameter("sel", [c * q, c], F32, isOutput=False)
    out = nc.declare_dram_parameter("out", [s], F32, isOutput=True)

    part_dram = nc.dram_tensor("part_bounce", [1, c], F32)
    # pair AllGather output: [even core's row | odd core's row] on one line
    full_dram = nc.dram_tensor("full_bounce", [1, 2 * c], F32)

    # DVE progress milestones (s_dve counts)
    DVE_PART = ntiles + 1  # per-partition totals ready
    DVE_ROW = ntiles + 2   # channel-sum row copied out of PSUM
    DVE_FULL = ntiles + 3  # pair-combined row in SBUF
    DVE_SUMS = ntiles + 4  # sums column in SBUF
    DVE_HAUG = ntiles + 5  # h_aug (post leaky relu) ready
    DVE_IDX = ntiles + 6   # ixv (selected channel index) ready

    with ExitStack() as ctx:
        ent = ctx.enter_context
        # SBUF
        xt = [
            ent(nc.sbuf_tensor(f"xt{i}", [128, tile_f], F32))
            for i in range(nslots)
        ]
        acc = ent(nc.sbuf_tensor([128, ntiles], F32))
        part = ent(nc.sbuf_tensor([128, 1], F32))
        row_sb = ent(nc.sbuf_tensor([1, c], F32))
        ag_sb = ent(nc.sbuf_tensor([1, 2 * c], F32))
        full_sb = ent(nc.sbuf_tensor([1, c], F32))
        sums_sb = ent(nc.sbuf_tensor([c, 1], F32))
        w1t_sb = ent(nc.sbuf_tensor([c, c], F32))
        b1_sb = ent(nc.sbuf_tensor([c, 1], F32))
        w2ta_sb = ent(nc.sbuf_tensor([c + 1, c], F32))
        rsel_sb = ent(nc.sbuf_tensor([1, 8], U32))
        sel_sb = ent(nc.sbuf_tensor([c * q, c], F32))
        h_pre = ent(nc.sbuf_tensor([c, 1], F32))
        h_scaled = ent(nc.sbuf_tensor([c, 1], F32))
        h_aug = ent(nc.sbuf_tensor([c + 1, 1], F32))
        s_sb = ent(nc.sbuf_tensor([1, c], F32))
        mx8 = ent(nc.sbuf_tensor([1, 8], F32))
        ix8 = ent(nc.sbuf_tensor([1, 8], U32))
        ixm = ent(nc.sbuf_tensor([1, 8], U32))
        ixv = ent(nc.sbuf_tensor([1, 1], U32))
        # PSUM
        row_ps = ent(nc.psum_tensor([1, c], F32))
        sums_ps = ent(nc.psum_tensor([c, 1], F32))
        ps1 = ent(nc.psum_tensor([c, 1], F32))
        ps2 = ent(nc.psum_tensor([1, c], F32))
        # semaphores
        s_slot = [ent(nc.semaphore(f"s_slot{i}")) for i in range(nslots)]
        s_dve = ent(nc.semaphore("s_dve"))
        s_pe = ent(nc.semaphore("s_pe"))
        s_wload = ent(nc.semaphore("s_wload"))
        s_part = ent(nc.semaphore("s_part"))
        s_cc = ent(nc.semaphore("s_cc"))
        s_sums = ent(nc.semaphore("s_sums"))
        s_out = ent(nc.semaphore("s_out"))
        block = ent(nc.Block())

        xg = x.rearrange("c (g f) -> c g f", g=2 * gather_chunks)
        og = out.rearrange("(g f) -> g f", g=2 * gather_chunks)

        def gather(eng, chunks):
            eng.wait_ge(s_dve, DVE_IDX)
            if static_gather_row is None:
                reg = nc.values_load(
                    ixv[0:1, 0:1], engines=[eng.engine],
                    min_val=0, max_val=c - 1, skip_runtime_bounds_check=True,
                )
                row = bass.ds(reg, 1)
            else:
                row = slice(static_gather_row, static_gather_row + 1)
            for g in chunks:
                eng.dma_start(
                    og[g : g + 1, :], xg[row, g : g + 1, :],
                    bounds_check=gather_bounds,
                ).then_inc(s_out, 16)

        @block.sync
        def _(sync: bass.BassEngine):
            # x tile loads, q-chunk 0, on the SP HWDGE ring
            for t in range(ntiles):
                slot = t % nslots
                if t >= nslots:
                    sync.wait_ge(s_dve, t - nslots + 1)
                lo = 0 * pp + t * tile_f
                sync.dma_start(
                    xt[slot][0:c, :], x[:, lo : lo + tile_f]
                ).then_inc(s_slot[slot], 16)
            if gather_engine == "rings":
                gather(sync, range(gather_chunks))
            sync.wait_ge(s_out, 16 * 2 * gather_chunks)

        @block.scalar
        def _(scalar: bass.BassEngine):
            # weight loads + x tile loads q-chunk 1, on the ACT HWDGE ring
            for t in range(ntiles):
                slot = t % nslots
                if t >= nslots:
                    scalar.wait_ge(s_dve, t - nslots + 1)
                lo = (q - 1) * pp + t * tile_f
                scalar.dma_start(
                    xt[slot][(q - 1) * c : q * c, :], x[:, lo : lo + tile_f]
                ).then_inc(s_slot[slot], 16)
            # weight loads after the x stream: they aren't consumed until the
            # MLP (PE gates on s_wload), and putting them first would delay
            # this ring's tile stream and the reduction end by their duration
            scalar.dma_start(w1t_sb[:], w1t[:]).then_inc(s_wload, 16)
            scalar.dma_start(b1_sb[:], b1[:]).then_inc(s_wload, 16)
            scalar.dma_start(w2ta_sb[:], w2ta[:]).then_inc(s_wload, 16)
            scalar.dma_start(rsel_sb[:], rsel[:]).then_inc(s_wload, 16)
            scalar.dma_start(sel_sb[:], sel[:]).then_inc(s_wload, 16)
            if gather_engine == "rings":
                gather(scalar, range(gather_chunks, 2 * gather_chunks))

        @block.gpsimd
        def _(gpsimd: bass.BassEngine):
            # channel-sum row exchange across the core pair (all single-line
            # contiguous 256 B DMAs)
            gpsimd.wait_ge(s_dve, DVE_ROW)
            gpsimd.dma_start(part_dram[:], row_sb[:]).then_inc(s_part, 16)
            if use_collective:
                # AllGather (not AllReduce: its naive cost is ~1.9x worse and
                # the pair-sum is one same-partition DVE add) -> [1, 2c] row
                gpsimd.wait_ge(s_part, 16)
                groups = [[i, i + 1] for i in range(0, n_cores, 2)]
                gpsimd.collective_compute(
                    "AllGather",
                    mybir.AluOpType.bypass,
                    replica_groups=groups,
                    ins=[part_dram[:]],
                    outs=[full_dram[:]],
                ).then_inc(s_cc, 1)
                gpsimd.wait_ge(s_cc, 1)
                gpsimd.dma_start(ag_sb[:], full_dram[:]).then_inc(s_sums, 16)
            else:
                gpsimd.wait_ge(s_part, 16)
                gpsimd.dma_start(
                    ag_sb[0:1, 0:c], part_dram[:]
                ).then_inc(s_sums, 16)
            if gather_engine == "gpsimd":
                gather(gpsimd, range(2 * gather_chunks))

        @block.vector
        def _(vector: bass.BassEngine):
            per_round = 16 * q  # sem value added to a slot sem per tile round
            for t in range(ntiles):
                slot = t % nslots
                vector.wait_ge(s_slot[slot], per_round * (t // nslots + 1))
                vector.reduce_sum(
                    acc[:, t : t + 1], xt[slot][:], axis=mybir.AxisListType.X
                ).then_inc(s_dve, 1)
            vector.drain()  # same-engine RAW: acc columns -> part reduce
            vector.reduce_sum(
                part[:], acc[:], axis=mybir.AxisListType.X
            ).then_inc(s_dve, 1)
            # channel-sum row out of PSUM (PE sel-matmul result)
            vector.wait_ge(s_pe, 1)
            vector.tensor_copy(row_sb[:], row_ps[:]).then_inc(s_dve, 1)
            # pair-sum of the AllGathered halves (same partition, free offsets)
            vector.wait_ge(s_sums, 16)
            if use_collective:
                vector.tensor_add(
                    full_sb[:], ag_sb[0:1, 0:c], ag_sb[0:1, c : 2 * c]
                )
            else:
                vector.tensor_copy(full_sb[:], ag_sb[0:1, 0:c])
            vector.drain().then_inc(s_dve, 1)
            # sums column out of PSUM (PE transpose result)
            vector.wait_ge(s_pe, 2)
            vector.tensor_copy(sums_sb[:], sums_ps[:]).then_inc(s_dve, 1)
            # MLP layer 1 epilogue: bias + leaky relu (ps1 from PE)
            vector.wait_ge(s_pe, 3)
            vector.tensor_add(h_pre[:], ps1[:], b1_sb[:])
            vector.drain()
            vector.tensor_scalar_mul(h_scaled[:], h_pre[:], LEAKY_SLOPE)
            vector.drain()
            vector.tensor_max(h_aug[0:c, :], h_pre[:], h_scaled[:])
            vector.memset(h_aug[c : c + 1, :], 1.0)
            vector.drain().then_inc(s_dve, 1)
            # layer 2 logits -> top-8 -> rank select (sigmoid is monotonic,
            # so pre-sigmoid logits rank identically)
            vector.wait_ge(s_pe, 4)
            vector.tensor_copy(s_sb[:], ps2[:])
            vector.drain()
            vector.max(mx8[:], s_sb[:])
            vector.drain()
            vector.max_index(ix8[:], mx8[:], s_sb[:])
            vector.drain()
            vector.tensor_tensor(
                ixm[:], ix8[:], rsel_sb[:], op=mybir.AluOpType.mult
            )
            vector.drain()
            with nc.allow_low_precision(reason="uint32 index add is exact"):
                vector.tensor_reduce(
                    ixv[:], ixm[:], axis=mybir.AxisListType.X,
                    op=mybir.AluOpType.add,
                ).then_inc(s_dve, 1)

        @block.tensor
        def _(tensor: bass.BassEngine):
            tensor.wait_ge(s_wload, 80)
            # part [128,1] -> channel-sum row [1, c] via selection matrix
            tensor.wait_ge(s_dve, DVE_PART)
            nc.tensor.matmul(
                row_ps[:], part[:], sel_sb[:], start=True, stop=True
            ).then_inc(s_pe, 1)
            # pair-combined row [1, c] -> column [c, 1] (sel[0,0] == 1.0)
            tensor.wait_ge(s_dve, DVE_FULL)
            nc.tensor.matmul(
                sums_ps[:], full_sb[:], sel_sb[0:1, 0:1], start=True, stop=True
            ).then_inc(s_pe, 1)
            # MLP matmuls
            tensor.wait_ge(s_dve, DVE_SUMS)
            nc.tensor.matmul(
                ps1[:], w1t_sb[:], sums_sb[:], start=True, stop=True
            ).then_inc(s_pe, 1)
            tensor.wait_ge(s_dve, DVE_HAUG)
            nc.tensor.matmul(
                ps2[:], h_aug[:], w2ta_sb[:], start=True, stop=True
            ).then_inc(s_pe, 1)

    return nc


def make_in_maps(x, W1, b1, W2, b2, c=C, s=S, n_cores=N_CORES, use_collective=True):
    """Shard full inputs into per-core input maps."""
    b_sz = x.shape[0]
    q = 128 // c
    x2 = np.ascontiguousarray(x.reshape(b_sz, c, s))
    w1t = np.ascontiguousarray(W1.T / np.float32(s)).astype(np.float32)
    b1c = np.ascontiguousarray(b1.reshape(c, 1)).astype(np.float32)
    w2ta = np.ascontiguousarray(
        np.concatenate([W2.T, b2[None, :]], axis=0)
    ).astype(np.float32)
    sel = np.tile(np.eye(c, dtype=np.float32), (q, 1))

    in_maps = []
    for i in range(n_cores):
        b_i, h_i = i // 2, i % 2
        xb = x2[b_i]
        if use_collective and h_i == 1:
            xb = np.roll(xb, -(s // 2), axis=1)
        rsel_i = np.zeros((1, 8), np.uint32)
        rsel_i[0, i % 2] = 1
        in_maps.append(
            {"x": np.ascontiguousarray(xb), "w1t": w1t, "b1": b1c,
             "w2ta": w2ta, "rsel": rsel_i, "sel": sel}
        )
    return in_maps


def assemble_output(results, b_sz=B, s=S, use_collective=True):
    """Reassemble per-core gathered rows into the full [B, R, D, H, W] output."""
    d = h = w = 64
    out = np.empty((b_sz, R, d, h, w), np.float32)
    for i, res in enumerate(results):
        b_i, r_i = i // 2, i % 2
        row = res["out"]
        if use_collective and i % 2 == 1:
            row = np.roll(row, s // 2)
        out[b_i, r_i] = row.reshape(d, h, w)
    return out


def kernel(x, W1, b1, W2, b2):
    x = np.asarray(x, dtype=np.float32)
    W1 = np.asarray(W1, dtype=np.float32)
    b1 = np.asarray(b1, dtype=np.float32)
    W2 = np.asarray(W2, dtype=np.float32)
    b2 = np.asarray(b2, dtype=np.float32)

    nc = build_bass()
    in_maps = make_in_maps(x, W1, b1, W2, b2)
    res = run_bass_kernel_spmd(nc, in_maps, list(range(N_CORES)))
    return assemble_output(res.results)


if __name__ == "__main__":
    rng = np.random.default_rng(0)
    x = rng.standard_normal((B, C, 64, 64, 64), dtype=np.float32)
    W1 = rng.standard_normal((C, C), dtype=np.float32) / np.sqrt(C)
    b1 = rng.standard_normal(C, dtype=np.float32) * 0.01
    W2 = rng.standard_normal((C, C), dtype=np.float32) / np.sqrt(C)
    b2 = rng.standard_normal(C, dtype=np.float32) * 0.01
    out = kernel(x=x, W1=W1, b1=b1, W2=W2, b2=b2)
    print(out.shape, out.dtype)

